# revision 69
# baseline (speedup 1.0000x reference)
"""Trainium2 Bass kernel for nn_DeformBottleneckBlock (v7).

kernel(**inputs) takes the full tensors of reference.setup_inputs() and
returns the full [2,1024,64,64] fp32 output. 8-way SPMD over NeuronCores,
H sharded 8 rows/core with a 5-row halo.

v7 changes vs the 346us baseline (now ~207us):
  - SWDGE descriptor carveout doubled (dynamic_dma_scratch_size=32768) so
    two gathers fit the ring: descgen of gather t+1 overlaps the DMA drain
    of gather t (was fully serialized at ~18us/tap).
  - num_idxs register hoisted out of the tap loop (the per-tap MOVE had a
    register WAR against the in-flight gather's DMA).
  - a dummy 128-idx gather right after init preloads the Pool SWDGE ucode
    library (~9us lib switch off the critical path).
  - index build: 128->16 partition fold via one-hot fp32 matmuls into two
    PSUM banks + one DVE permute-cast (replaces 8 SWDGE 2-byte element
    scatters whose tiny-packet DMA drain took ~15us).
  - pixel-half (== batch) software pipeline in the deform phase: taps for
    batch 0 (512-idx gathers), then batch 0's conv3+residual+store overlap
    batch 1's gathers. Broadcast weight rows prefetched per half.
  - x tiles live only through phase 1; the residual re-loads just the
    center pixels from HBM during phase 6; output stored as fp16.
  - idx broadcasts and output DMAs alternate the sync/scalar HWDGE queues.
"""

import numpy as np
from contextlib import ExitStack

B, CIN, H, W = 2, 1024, 64, 64
CB, COUT = 256, 1024
NCORES = 8
RPC = H // NCORES          # 8 output rows per core
MH = 5                     # halo rows (covers |offset| <= 2.9)
RS = RPC + 2 * MH          # 18 slab rows
WP = W + 2                 # 66 padded cols
SLABPIX = B * RS * WP      # 2376
NPIX = B * RPC * W         # 1024 output pixels per core
NG = NPIX // 128           # 8 pixel groups
EPS = 1e-5
NIDX = 9 * NPIX            # 9216 gather indices (one per pixel*tap)
NGT = (SLABPIX + 127) // 128   # transpose chunks (19, last partial=72)
# o1t2 tokens: t = 2*q + 132 (write1) max q=2375 -> 4882; alloc 4884
O1TOK = 2 * (SLABPIX - 1) + 132 + 2   # 4884


def build_nc():
    import concourse.bass as bass
    import concourse.mybir as mybir
    import concourse.tile as tile
    from concourse import bacc
    from concourse.tile import add_dep_helper
    from concourse.masks import make_identity

    F16 = mybir.dt.float16
    F32 = mybir.dt.float32
    I16 = mybir.dt.int16
    AF = mybir.ActivationFunctionType
    ALU = mybir.AluOpType

    nc = bacc.Bacc(None, target_bir_lowering=False, debug=False,
                   num_swdge_queues=1, dynamic_dma_scratch_size=32768)

    xs = nc.declare_dram_parameter("xs", [128, 8, SLABPIX], F16, isOutput=False)
    msk = nc.declare_dram_parameter("msk", [1, SLABPIX], F16, isOutput=False)
    w1 = nc.declare_dram_parameter("w1", [128, 8, 256], F16, isOutput=False)
    b1 = nc.declare_dram_parameter("b1", [1, 256], F16, isOutput=False)
    woff = nc.declare_dram_parameter("woff", [128, 9, 2, 18], F16, isOutput=False)
    boff = nc.declare_dram_parameter("boff", [18, 1], F32, isOutput=False)
    w2 = nc.declare_dram_parameter("w2", [128, 18, 256], F16, isOutput=False)
    b2 = nc.declare_dram_parameter("b2", [128, 2, 1], F32, isOutput=False)
    w3 = nc.declare_dram_parameter("w3", [128, 2, 1024], F16, isOutput=False)
    b3 = nc.declare_dram_parameter("b3", [128, 8, 1], F32, isOutput=False)
    gy = nc.declare_dram_parameter("gy", [128, NG, 9], F32, isOutput=False)
    gx = nc.declare_dram_parameter("gx", [128, NG, 9], F32, isOutput=False)
    gq = nc.declare_dram_parameter("gq", [128, NG, 9], F32, isOutput=False)
    sel = nc.declare_dram_parameter("sel", [36, 36, 128], F16, isOutput=False)
    esel = nc.declare_dram_parameter("esel", [128, 8, 16], F32, isOutput=False)
    outp = nc.declare_dram_parameter("out", [8, 128, NPIX], F16, isOutput=True)

    o1t2 = nc.dram_tensor("o1t2", [O1TOK, 256], F16)

    with ExitStack() as ctx:
        tc = ctx.enter_context(tile.TileContext(nc))

        const = ctx.enter_context(tc.tile_pool(name="const", bufs=1))
        wk = ctx.enter_context(tc.tile_pool(name="wk", bufs=1))
        small = ctx.enter_context(tc.tile_pool(name="small", bufs=1))
        opool = ctx.enter_context(tc.tile_pool(name="opool", bufs=2))

        # ---- constants (w1/b1/msk on scalar queue; x tiles stream on sync
        #      so the first conv1 matmul can start as early as possible)
        w1_sb = const.tile([128, 8, 256], F16)
        nc.scalar.dma_start(out=w1_sb, in_=w1[:])
        b1_sb = const.tile([1, 256], F16)
        nc.scalar.dma_start(out=b1_sb, in_=b1[:])
        msk_sb = const.tile([1, SLABPIX], F16)
        nc.scalar.dma_start(out=msk_sb, in_=msk[:])

        # x tiles live only through phase 1 (residual reloads from HBM)
        xpool_cm = tc.tile_pool(name="xpool", bufs=1)
        xpool = xpool_cm.__enter__()
        x_t = []
        for kc in range(8):
            xt = xpool.tile([128, SLABPIX], F16, tag=f"x{kc}", name=f"x{kc}")
            nc.sync.dma_start(out=xt, in_=xs[:, kc, :])
            x_t.append(xt)

        woff_sb = const.tile([128, 9, 2, 18], F16)
        nc.scalar.dma_start(out=woff_sb, in_=woff[:])
        boff_sb = const.tile([18, 1], F32)
        nc.scalar.dma_start(out=boff_sb, in_=boff[:])
        w2_sb = const.tile([128, 18, 256], F16)
        nc.scalar.dma_start(out=w2_sb, in_=w2[:])
        b2_sb = const.tile([128, 2, 1], F32)
        nc.scalar.dma_start(out=b2_sb, in_=b2[:])
        w3_sb = const.tile([128, 2, 1024], F16)
        nc.scalar.dma_start(out=w3_sb, in_=w3[:])
        b3_sb = const.tile([128, 8, 1], F32)
        nc.scalar.dma_start(out=b3_sb, in_=b3[:])
        gy_sb = const.tile([128, NG, 9], F32)
        nc.scalar.dma_start(out=gy_sb, in_=gy[:])
        gx_sb = const.tile([128, NG, 9], F32)
        nc.scalar.dma_start(out=gx_sb, in_=gx[:])
        gq_sb = const.tile([128, NG, 9], F32)
        nc.scalar.dma_start(out=gq_sb, in_=gq[:])
        id16 = const.tile([128, 128], F16)
        make_identity(nc, id16)
        id32 = const.tile([128, 128], F32)
        make_identity(nc, id32)
        sel_sb = const.tile([36, 36, 128], F16)
        nc.scalar.dma_start(out=sel_sb, in_=sel[:])
        esel_sb = const.tile([128, 8, 16], F32)
        nc.scalar.dma_start(out=esel_sb, in_=esel[:])

        DUMMY_GATHER = True
        if DUMMY_GATHER:
            # dummy 128-idx gather: forces the Pool SWDGE library load early
            idx0 = const.tile([16, 8], I16)
            nc.gpsimd.memset(idx0[:], 0)
            with tc.tile_pool(name="gdum", bufs=1) as gdp:
                gdum = gdp.tile([128, 2, 128], F16)
                dum_ap = bass.AP(tensor=o1t2[:].tensor, offset=0,
                                 ap=[[256, 64], [1, 256]])
                nc.gpsimd.dma_gather(
                    out_ap=gdum, in_ap=dum_ap, idxs_ap=idx0,
                    num_idxs=128, num_idxs_reg=128,
                    elem_size=256, elem_step=256, transpose=True,
                    single_packet=False)

        # whole-kernel working tensors
        out1_sb = wk.tile([128, 2, SLABPIX], F16)
        offT = wk.tile([128, NG, 18], F32)
        W4 = wk.tile([128, NG, 9, 4], F32)
        W4C = wk.tile([36, NPIX], F16)
        idx16 = wk.tile([16, NIDX // 16], I16)
        idx_sb = wk.tile([128, NIDX // 16], I16)
        stg = wk.tile([128, NGT, 256], F16)

        # o1t2 token writes: row y lands twice (s=0 of block y+1: t=2q+132;
        # s=1 of block y: t=2q+1). Split per half so writes start early.
        NFULL = SLABPIX // 128          # 18 full transpose chunks
        TAILW = SLABPIX - NFULL * 128   # 72
        o1flat = o1t2[:].rearrange("t c -> (t c)")
        wr_insts = []

        def o1t2_write(c0, c1, eng):
            for toff in (132 * 256, 256):
                dstm = bass.AP(tensor=o1flat.tensor,
                               offset=toff + 512 * 128 * c0,
                               ap=[[512, 128], [512 * 128, c1 - c0], [1, 256]])
                wr = eng.dma_start(out=dstm, in_=stg[:, c0:c1, :])
                wr_insts.append(wr.ins)

        def o1t2_tail(eng):
            for toff in (132 * 256, 256):
                dstt = bass.AP(tensor=o1flat.tensor,
                               offset=toff + 512 * NFULL * 128,
                               ap=[[512, TAILW], [1, 256]])
                wr = eng.dma_start(out=dstt, in_=stg[0:TAILW, NFULL, :])
                wr_insts.append(wr.ins)

        def eng_of(i):
            return nc.sync if i % 2 == 0 else nc.scalar

        # ---- phase 1: conv1x1 (1024->256) + BN + ReLU, DMA-pipelined,
        #      interleaved with phase 5 transposes (o1t2 staging)
        NCH = 6
        CW = SLABPIX // NCH  # 396
        with tc.tile_pool(name="psA", bufs=1, space="PSUM") as psA, \
             tc.tile_pool(name="ps5", bufs=2, space="PSUM") as ps5:
            for half in range(2):
                pss = [psA.tile([128, CW], F32, tag=f"c1_{i}",
                                name=f"psc1_{i}") for i in range(6)]
                for kc in range(8):
                    xsb = x_t[kc]
                    for m in range(2):
                        for i in range(3):
                            nch = half * 3 + i
                            sl = slice(nch * CW, (nch + 1) * CW)
                            nc.tensor.matmul(
                                pss[m * 3 + i],
                                lhsT=w1_sb[:, kc, m * 128:(m + 1) * 128],
                                rhs=xsb[:, sl],
                                start=(kc == 0), stop=False)
                for m in range(2):
                    for i in range(3):
                        nch = half * 3 + i
                        sl = slice(nch * CW, (nch + 1) * CW)
                        nc.tensor.matmul(
                            pss[m * 3 + i],
                            lhsT=b1_sb[:1, m * 128:(m + 1) * 128],
                            rhs=msk_sb[:1, sl], start=False, stop=True)
                        nc.scalar.activation(out1_sb[:, m, sl],
                                             pss[m * 3 + i], AF.Relu)
                # phase 5 transposes for the chunks this half completed
                clo = 0 if half == 0 else 9
                chi = 9 if half == 0 else NGT
                for g in range(clo, chi):
                    wc = min(128, SLABPIX - g * 128)
                    pt = ps5.tile([128, 256], F16, tag="t16", name="pt5")
                    for kc in range(2):
                        nc.tensor.transpose(
                            pt[:wc, kc * 128:(kc + 1) * 128],
                            out1_sb[:, kc, g * 128:g * 128 + wc], id16)
                    nc.vector.tensor_copy(stg[:wc, g, :], pt[:wc, :])
                if half == 0:
                    o1t2_write(0, 9, nc.sync)
                else:
                    o1t2_write(9, NFULL, nc.sync)
                    o1t2_tail(nc.sync)

        xpool_cm.__exit__(None, None, None)

        # ---- phase 2: offset conv3x3 over output rows -> offs_sb [18,1024]
        with tc.tile_pool(name="bpool", bufs=1) as bpool:
            offs_sb = bpool.tile([18, 1024], F32)
            with tc.tile_pool(name="psB", bufs=2, space="PSUM") as psB:
                for b in range(B):
                    for hh in range(2):
                        base = (b * RS + MH) * WP + hh * 264
                        ps = psB.tile([18, 264], F32, tag="off", name="psoff")
                        first = True
                        for t in range(9):
                            tau = (t // 3 - 1) * WP + (t % 3 - 1)
                            for kc in range(2):
                                nc.tensor.matmul(
                                    ps, lhsT=woff_sb[:, t, kc, :],
                                    rhs=out1_sb[:, kc,
                                                base + tau:base + tau + 264],
                                    start=first,
                                    stop=(t == 8 and kc == 1))
                                first = False
                        dst = offs_sb[:, (b * 2 + hh) * 256:
                                      (b * 2 + hh + 1) * 256].rearrange(
                            "p (r c) -> p r c", c=64)
                        src = ps.rearrange("p (r c) -> p r c",
                                           c=WP)[:, :, 1:65]
                        nc.scalar.activation(dst, src, AF.Identity,
                                             bias=boff_sb)

            # ---- phase 3: offsets -> pixel-major [128, NG, 18]
            with tc.tile_pool(name="ps3", bufs=2, space="PSUM") as ps3:
                for g in range(NG):
                    p32 = ps3.tile([128, 18], F32, tag="t32", name="p32")
                    nc.tensor.transpose(
                        p32, offs_sb[:, g * 128:(g + 1) * 128],
                        id32[:18, :18])
                    nc.vector.tensor_copy(offT[:, g, :], p32)

        # ---- phase 4: coords, weights, gather indices
        oy = offT[:, :, 0:9]
        ox = offT[:, :, 9:18]

        def stile(tag):
            return small.tile([128, NG, 9], F32, tag=tag, name=tag)

        I32 = mybir.dt.int32

        def floorsplit(p, pfx, eng):
            """Exact floor via cast roundtrip + negative-error fixup."""
            ii = small.tile([128, NG, 9], I32, tag=pfx + "i", name=pfx + "i")
            eng.tensor_copy(ii, p)
            fcast = small.tile([128, NG, 9], F32, tag=pfx + "c", name=pfx + "c")
            eng.tensor_copy(fcast, ii)
            d = small.tile([128, NG, 9], F32, tag=pfx + "d", name=pfx + "d")
            eng.tensor_tensor(d, p, fcast, ALU.subtract)
            mk = small.tile([128, NG, 9], F32, tag=pfx + "m", name=pfx + "m")
            eng.tensor_scalar(mk, d, 0.0, None, ALU.is_lt)
            fl = small.tile([128, NG, 9], F32, tag=pfx + "f", name=pfx + "f")
            eng.tensor_tensor(fl, fcast, mk, ALU.subtract)
            fr = small.tile([128, NG, 9], F32, tag=pfx + "r", name=pfx + "r")
            eng.tensor_tensor(fr, p, fl, ALU.subtract)
            return fl, fr

        # py in batch-relative slab rows [0, RS): clamp [0, RS-2.1]
        # y path on vector, x path on gpsimd (runs before gathers start)
        py = stile("py")
        nc.vector.tensor_tensor(py, oy, gy_sb, ALU.add)
        nc.vector.tensor_scalar(py, py, 0.0, float(RS) - 2.1, ALU.max, ALU.min)
        y0f, fy = floorsplit(py, "y", nc.vector)
        px = stile("px")
        nc.vector.tensor_tensor(px, ox, gx_sb, ALU.add)
        nc.vector.tensor_scalar(px, px, 0.0, 65.9, ALU.max, ALU.min)
        x0f, fx = floorsplit(px, "x", nc.vector)

        u = stile("u")
        nc.vector.tensor_scalar(u, fy, -1.0, 1.0, ALU.mult, ALU.add)
        v = stile("v")
        nc.vector.tensor_scalar(v, fx, -1.0, 1.0, ALU.mult, ALU.add)
        nc.vector.tensor_tensor(W4[:, :, :, 0], u, v, ALU.mult)
        nc.vector.tensor_tensor(W4[:, :, :, 1], u, fx, ALU.mult)
        nc.vector.tensor_tensor(W4[:, :, :, 2], fy, v, ALU.mult)
        nc.vector.tensor_tensor(W4[:, :, :, 3], fy, fx, ALU.mult)

        # patch token index: q = 132*y0 + 2*x0 + (2376*b + 132)
        qf = stile("qf")
        nc.vector.scalar_tensor_tensor(qf, y0f, 132.0, gq_sb,
                                       ALU.mult, ALU.add)
        qf2 = stile("qf2")
        nc.vector.scalar_tensor_tensor(qf2, x0f, 2.0, qf, ALU.mult, ALU.add)

        # partition fold 128->16 via one-hot fp32 matmuls (exact for ints):
        # psI[mm, 72a + 9g + k] = qf2[16a+mm, g, k]; then one DVE permute-
        # cast to idx16[mm, 64k + 8g + a] and HWDGE broadcasts to idx_sb.
        with tc.tile_pool(name="psI", bufs=2, space="PSUM") as psIp:
            qf2f = qf2.rearrange("p g k -> p (g k)")
            idx4 = idx16.rearrange("m (k g a) -> m k g a", k=9, g=8, a=8)
            for h in range(2):
                psI = psIp.tile([16, 288], F32, tag="psI", name=f"psI{h}")
                for a4 in range(4):
                    nc.tensor.matmul(psI[:, a4 * 72:(a4 + 1) * 72],
                                     lhsT=esel_sb[:, h * 4 + a4, :],
                                     rhs=qf2f, start=True, stop=True)
                nc.vector.tensor_copy(
                    idx4[:, :, :, h * 4:(h + 1) * 4],
                    psI.rearrange("m (a g k) -> m k g a", a=4, g=8, k=9))
        for bb in range(8):
            eng_of(bb).dma_start(out=idx_sb[16 * bb:16 * bb + 16, :],
                                 in_=idx16)

        # W4 -> channel-major fp16 rows W4C[(t*4+nb), pix] for broadcast
        with tc.tile_pool(name="ps4", bufs=2, space="PSUM") as ps4:
            for g in range(NG):
                p36 = ps4.tile([36, 128], F32, tag="w4c", name="p36")
                nc.tensor.transpose(
                    p36, W4[:, g, :, :].rearrange("p a b -> p (a b)"), id32)
                nc.scalar.copy(W4C[:, g * 128:(g + 1) * 128], p36)

        # residual pixels preloaded from xs HBM (overlaps phase 6)
        xr_t = []
        for m in range(8):
            xsrc4 = xs[:, m, :].rearrange("p (b r c) -> p b r c", r=RS, c=WP)
            xr = wk.tile([128, B, RPC, 64], F16, tag=f"xr{m}", name=f"xr{m}")
            for b in range(B):
                eng_of(m + b).dma_start(
                    out=xr[:, b], in_=xsrc4[:, b, MH:MH + RPC, 1:65])
            xr_t.append(xr)

        # ---- phase 6: transposed patch gather (channel-major) + weighting
        src_ap = bass.AP(tensor=o1t2[:].tensor, offset=0,
                         ap=[[256, O1TOK - 3], [1, 1024]])
        psd_cm = tc.tile_pool(name="psd", bufs=2, space="PSUM")
        with tc.tile_pool(name="gpool", bufs=3) as gpool, \
             tc.tile_pool(name="wpool", bufs=3) as wpool, \
             tc.tile_pool(name="wps", bufs=2, space="PSUM") as wps, \
             tc.tile_pool(name="accp", bufs=1) as accp, \
             tc.tile_pool(name="spool", bufs=2) as spool, \
             psd_cm as psd:
            out2_sb = wk.tile([128, 2, NPIX], F16)

            # pixel-half (== batch) software pipeline: half-0 taps, then
            # half-0 conv3/out overlapped with half-1 taps. Broadcast weight
            # rows (sel-matmul + ACT evac) are prefetched per half.
            nidx_reg = nc.gpsimd.to_reg(512)
            with tc.tile_pool(name="psC", bufs=2, space="PSUM") as psC:
                for h in range(2):
                    hs = slice(h * 512, (h + 1) * 512)
                    wB_t = []
                    for t in range(9):
                        wB = wpool.tile([128, 4, 512], F16, tag="wB",
                                        name=f"wB{h}_{t}")
                        for nb in range(4):
                            wp = wps.tile([128, 512], F32, tag="wp",
                                          name="wp")
                            nc.tensor.matmul(
                                wp, lhsT=sel_sb[:, 4 * t + nb, :],
                                rhs=W4C[:, hs], start=True, stop=True)
                            nc.scalar.copy(wB[:, nb, :], wp)
                        wB_t.append(wB)
                    dps = psd.tile([128, 2, 512], F32, tag="dps", name="dps")
                    for t in range(9):
                        wB = wB_t[t]
                        g_t = gpool.tile([128, 8, 512], F16, tag="g",
                                         name="g_t")
                        gi = nc.gpsimd.dma_gather(
                            out_ap=g_t, in_ap=src_ap,
                            idxs_ap=idx_sb[:, t * 64 + 32 * h:
                                           t * 64 + 32 * h + 32],
                            num_idxs=512, num_idxs_reg=nidx_reg,
                            elem_size=1024, elem_step=256, transpose=True,
                            single_packet=False)
                        for wi in wr_insts:
                            add_dep_helper(gi.ins, wi,
                                           reason="gather after o1t2")

                        # chunks: (0,1)=y0x0 (2,3)=y1x0 (4,5)=y0x1 (6,7)=y1x1
                        # nb: 0=u*v(y0x0) 1=u*fx(y0x1) 2=fy*v(y1x0) 3=fy*fx
                        def wv(nb):
                            return wB[:, nb:nb + 1, :].broadcast_to(
                                [128, 2, 512])

                        ta = accp.tile([128, 2, 512], F16, tag="ta",
                                       name="ta")
                        nc.vector.tensor_tensor(ta, g_t[:, 0:2, :], wv(0),
                                                ALU.mult)
                        tb = accp.tile([128, 2, 512], F16, tag="tb",
                                       name="tb")
                        nc.vector.tensor_tensor(tb, g_t[:, 2:4, :], wv(2),
                                                ALU.mult)
                        tab = accp.tile([128, 2, 512], F16, tag="tab",
                                        name="tab")
                        nc.vector.tensor_tensor(tab, ta, tb, ALU.add)
                        tc_ = accp.tile([128, 2, 512], F16, tag="ta",
                                        name="tc_")
                        nc.vector.tensor_tensor(tc_, g_t[:, 4:6, :], wv(1),
                                                ALU.mult)
                        td = accp.tile([128, 2, 512], F16, tag="tb",
                                       name="td")
                        nc.vector.tensor_tensor(td, g_t[:, 6:8, :], wv(3),
                                                ALU.mult)
                        tcd = accp.tile([128, 2, 512], F16, tag="tcd",
                                        name="tcd")
                        nc.vector.tensor_tensor(tcd, tc_, td, ALU.add)
                        S = spool.tile([128, 2, 512], F16, tag="S", name="S")
                        nc.vector.tensor_tensor(S, tab, tcd, ALU.add)

                        for ch in range(2):
                            j = 2 * t + ch
                            for m in range(2):
                                nc.tensor.matmul(
                                    dps[:, m, :],
                                    lhsT=w2_sb[:, j, m * 128:(m + 1) * 128],
                                    rhs=S[:, ch, :],
                                    start=(t == 0 and ch == 0),
                                    stop=(t == 8 and ch == 1))

                    # deform psum evac for this half: BN + ReLU
                    for m2 in range(2):
                        nc.scalar.activation(out2_sb[:, m2, hs],
                                             dps[:, m2, :],
                                             AF.Relu, bias=b2_sb[:, m2, :])

                    # conv3 + residual + out for this half (overlaps the
                    # other half's gathers)
                    for m in range(8):
                        ps = psC.tile([128, 512], F32, tag="c3", name="psc3")
                        for kc in range(2):
                            nc.tensor.matmul(
                                ps,
                                lhsT=w3_sb[:, kc, m * 128:(m + 1) * 128],
                                rhs=out2_sb[:, kc, hs],
                                start=(kc == 0), stop=(kc == 1))
                        xv = xr_t[m].rearrange("p b r c -> p (b r c)")[:, hs]
                        rt = opool.tile([128, 512], F32, tag="res",
                                        name="rt")
                        nc.vector.tensor_tensor(rt, ps, xv, ALU.add)
                        ot = opool.tile([128, 512], F16, tag="out",
                                        name="ot")
                        nc.scalar.activation(ot, rt, AF.Relu,
                                             bias=b3_sb[:, m, :])
                        eng_of(m).dma_start(out=outp[m][:, hs], in_=ot)

    nc.compile()
    return nc


# ---------------------------------------------------------------- host side
def fold_weights(inputs):
    f = {}
    w1, g1, b1, m1, v1 = (np.asarray(inputs[k], np.float64)
                          for k in ("w1", "g1", "b1", "m1", "v1"))
    s1 = g1 / np.sqrt(v1 + EPS)
    W1f = w1[:, :, 0, 0] * s1[:, None]
    f["w1"] = np.ascontiguousarray(
        W1f.T.reshape(8, 128, 256).transpose(1, 0, 2)).astype(np.float16)
    f["b1"] = (b1 - m1 * s1).reshape(1, 256).astype(np.float16)

    w_off = np.asarray(inputs["w_off"], np.float64)
    b_off = np.asarray(inputs["b_off"], np.float64)
    perm = np.concatenate([np.arange(0, 18, 2), np.arange(1, 18, 2)])
    wofft = np.zeros((9, 2, 128, 18), np.float16)
    for t in range(9):
        wk = w_off[perm, :, t // 3, t % 3].T  # [256, 18]
        wofft[t] = wk.reshape(2, 128, 18).astype(np.float16)
    f["woff"] = np.ascontiguousarray(wofft.transpose(2, 0, 1, 3))
    f["boff"] = b_off[perm].reshape(18, 1).astype(np.float32)

    w2, g2, b2, m2, v2 = (np.asarray(inputs[k], np.float64)
                          for k in ("w2", "g2", "b2", "m2", "v2"))
    s2 = g2 / np.sqrt(v2 + EPS)
    W2f = w2 * s2[:, None, None, None]
    w2r = np.zeros((18, 128, 256), np.float16)
    for t in range(9):
        for ch in range(2):
            w2r[2 * t + ch] = W2f[:, ch * 128:(ch + 1) * 128,
                                  t // 3, t % 3].T.astype(np.float16)
    f["w2"] = np.ascontiguousarray(w2r.transpose(1, 0, 2))
    f["b2"] = np.ascontiguousarray(
        (b2 - m2 * s2).reshape(2, 128, 1).transpose(1, 0, 2)).astype(np.float32)

    w3, g3, b3, m3, v3 = (np.asarray(inputs[k], np.float64)
                          for k in ("w3", "g3", "b3", "m3", "v3"))
    s3 = g3 / np.sqrt(v3 + EPS)
    W3f = w3[:, :, 0, 0] * s3[:, None]
    f["w3"] = np.ascontiguousarray(
        W3f.T.reshape(2, 128, 1024).transpose(1, 0, 2)).astype(np.float16)
    f["b3"] = np.ascontiguousarray(
        (b3 - m3 * s3).reshape(8, 128, 1).transpose(1, 0, 2)).astype(np.float32)
    return f


def make_grids():
    p = np.arange(NPIX)
    b = p // (RPC * W)
    r = (p % (RPC * W)) // W
    c = p % W
    kdy = np.arange(9) // 3 - 1
    kdx = np.arange(9) % 3 - 1
    gy = (r + MH)[:, None] + kdy[None, :]          # batch-relative slab row
    gx = (c + 1)[:, None] + kdx[None, :]
    gq = np.broadcast_to((SLABPIX // B * 2 * b + 132)[:, None],
                         gy.shape).copy()          # 2376*b + 132

    def shape(a):
        return np.ascontiguousarray(
            a.reshape(NG, 128, 9).transpose(1, 0, 2)).astype(np.float32)
    return shape(gy), shape(gx), shape(gq)


def make_core_inputs(x, folded, grids, core):
    row0 = RPC * core - MH
    xp = np.zeros((B, CIN, RS, WP), np.float16)
    lo, hi = max(0, row0), min(H, row0 + RS)
    if hi > lo:
        xp[:, :, lo - row0:hi - row0, 1:65] = x[:, :, lo:hi, :].astype(np.float16)
    xs = np.ascontiguousarray(
        xp.transpose(1, 0, 2, 3).reshape(8, 128, SLABPIX).transpose(1, 0, 2))

    mk = np.zeros((B, RS, WP), np.float16)
    mk[:, lo - row0:hi - row0, 1:65] = 1.0

    gy, gx, gq = grids
    selmat = np.zeros((36, 36, 128), np.float16)
    for k in range(36):
        selmat[k, k, :] = 1.0
    eselmat = np.zeros((128, 8, 16), np.float32)
    for a in range(8):
        for mm in range(16):
            eselmat[16 * a + mm, a, mm] = 1.0
    m = dict(folded)
    m.update(xs=xs, msk=mk.reshape(1, SLABPIX), gy=gy, gx=gx, gq=gq,
             sel=selmat, esel=eselmat)
    return m


def assemble_output(results):
    full = np.zeros((B, COUT, H, W), np.float32)
    for core, res in enumerate(results):
        o = np.asarray(res["out"]).reshape(8, 128, B, RPC, W)
        full[:, :, RPC * core:RPC * (core + 1), :] = o.transpose(
            2, 0, 1, 3, 4).reshape(B, COUT, RPC, W)
    return full


_NC_CACHE = None


def kernel(**inputs):
    global _NC_CACHE
    from concourse.bass_utils import run_bass_kernel_spmd

    x = np.asarray(inputs["x"], np.float32)
    folded = fold_weights(inputs)
    grids = make_grids()
    in_maps = [make_core_inputs(x, folded, grids, i) for i in range(NCORES)]

    if _NC_CACHE is None:
        _NC_CACHE = build_nc()
    res = run_bass_kernel_spmd(_NC_CACHE, in_maps, list(range(NCORES)))
    return assemble_output(res.results)



# revision 72
# speedup vs baseline: 1.0868x; 1.0868x over previous
"""Trainium2 Bass kernel for nn_DeformBottleneckBlock (v7).

kernel(**inputs) takes the full tensors of reference.setup_inputs() and
returns the full [2,1024,64,64] fp32 output. 8-way SPMD over NeuronCores,
H sharded 8 rows/core with a 5-row halo.

v7 changes vs the 346us baseline (now ~207us):
  - SWDGE descriptor carveout doubled (dynamic_dma_scratch_size=32768) so
    two gathers fit the ring: descgen of gather t+1 overlaps the DMA drain
    of gather t (was fully serialized at ~18us/tap).
  - num_idxs register hoisted out of the tap loop (the per-tap MOVE had a
    register WAR against the in-flight gather's DMA).
  - a dummy 128-idx gather right after init preloads the Pool SWDGE ucode
    library (~9us lib switch off the critical path).
  - index build: 128->16 partition fold via one-hot fp32 matmuls into two
    PSUM banks + one DVE permute-cast (replaces 8 SWDGE 2-byte element
    scatters whose tiny-packet DMA drain took ~15us).
  - pixel-half (== batch) software pipeline in the deform phase: taps for
    batch 0 (512-idx gathers), then batch 0's conv3+residual+store overlap
    batch 1's gathers. Broadcast weight rows prefetched per half.
  - x tiles live only through phase 1; the residual re-loads just the
    center pixels from HBM during phase 6; output stored as fp16.
  - idx broadcasts and output DMAs alternate the sync/scalar HWDGE queues.
"""

import numpy as np
from contextlib import ExitStack

B, CIN, H, W = 2, 1024, 64, 64
CB, COUT = 256, 1024
NCORES = 8
RPC = H // NCORES          # 8 output rows per core
MH = 5                     # halo rows (covers |offset| <= 2.9)
RS = RPC + 2 * MH          # 18 slab rows
WP = W + 2                 # 66 padded cols
SLABPIX = B * RS * WP      # 2376
NPIX = B * RPC * W         # 1024 output pixels per core
NG = NPIX // 128           # 8 pixel groups
EPS = 1e-5
NIDX = 9 * NPIX            # 9216 gather indices (one per pixel*tap)
NGT = (SLABPIX + 127) // 128   # transpose chunks (19, last partial=72)
# o1t2 tokens: t = 2*q + 132 (write1) max q=2375 -> 4882; alloc 4884
O1TOK = 2 * (SLABPIX - 1) + 132 + 2   # 4884


def build_nc():
    import concourse.bass as bass
    import concourse.mybir as mybir
    import concourse.tile as tile
    from concourse import bacc
    from concourse.tile import add_dep_helper
    from concourse.masks import make_identity

    F16 = mybir.dt.float16
    F32 = mybir.dt.float32
    I16 = mybir.dt.int16
    AF = mybir.ActivationFunctionType
    ALU = mybir.AluOpType

    nc = bacc.Bacc(None, target_bir_lowering=False, debug=False,
                   num_swdge_queues=1, dynamic_dma_scratch_size=32768)

    xs = nc.declare_dram_parameter("xs", [128, 8, SLABPIX], F16, isOutput=False)
    msk = nc.declare_dram_parameter("msk", [1, SLABPIX], F16, isOutput=False)
    w1 = nc.declare_dram_parameter("w1", [128, 8, 256], F16, isOutput=False)
    b1 = nc.declare_dram_parameter("b1", [1, 256], F16, isOutput=False)
    woff = nc.declare_dram_parameter("woff", [128, 9, 2, 18], F16, isOutput=False)
    boff = nc.declare_dram_parameter("boff", [18, 1], F32, isOutput=False)
    w2 = nc.declare_dram_parameter("w2", [128, 18, 256], F16, isOutput=False)
    b2 = nc.declare_dram_parameter("b2", [128, 2, 1], F32, isOutput=False)
    w3 = nc.declare_dram_parameter("w3", [128, 2, 1024], F16, isOutput=False)
    b3 = nc.declare_dram_parameter("b3", [128, 8, 1], F32, isOutput=False)
    gy = nc.declare_dram_parameter("gy", [128, NG, 9], F32, isOutput=False)
    gx = nc.declare_dram_parameter("gx", [128, NG, 9], F32, isOutput=False)
    gq = nc.declare_dram_parameter("gq", [128, NG, 9], F32, isOutput=False)
    sel = nc.declare_dram_parameter("sel", [36, 36, 128], F16, isOutput=False)
    esel = nc.declare_dram_parameter("esel", [128, 8, 16], F32, isOutput=False)
    outp = nc.declare_dram_parameter("out", [8, 128, NPIX], F16, isOutput=True)

    o1t2 = nc.dram_tensor("o1t2", [O1TOK, 256], F16)

    with ExitStack() as ctx:
        tc = ctx.enter_context(tile.TileContext(nc))

        const = ctx.enter_context(tc.tile_pool(name="const", bufs=1))
        wk = ctx.enter_context(tc.tile_pool(name="wk", bufs=1))
        small = ctx.enter_context(tc.tile_pool(name="small", bufs=1))
        opool = ctx.enter_context(tc.tile_pool(name="opool", bufs=2))

        # ---- constants (w1/b1/msk on scalar queue; x tiles stream on sync
        #      so the first conv1 matmul can start as early as possible)
        w1_sb = const.tile([128, 8, 256], F16)
        nc.scalar.dma_start(out=w1_sb, in_=w1[:])
        b1_sb = const.tile([1, 256], F16)
        nc.scalar.dma_start(out=b1_sb, in_=b1[:])
        msk_sb = const.tile([1, SLABPIX], F16)
        nc.scalar.dma_start(out=msk_sb, in_=msk[:])

        # x tiles live only through phase 1 (residual reloads from HBM)
        xpool_cm = tc.tile_pool(name="xpool", bufs=1)
        xpool = xpool_cm.__enter__()
        x_t = []
        for kc in range(8):
            xt = xpool.tile([128, SLABPIX], F16, tag=f"x{kc}", name=f"x{kc}")
            nc.sync.dma_start(out=xt, in_=xs[:, kc, :])
            x_t.append(xt)

        woff_sb = const.tile([128, 9, 2, 18], F16)
        nc.scalar.dma_start(out=woff_sb, in_=woff[:])
        boff_sb = const.tile([18, 1], F32)
        nc.scalar.dma_start(out=boff_sb, in_=boff[:])
        w2_sb = const.tile([128, 18, 256], F16)
        nc.scalar.dma_start(out=w2_sb, in_=w2[:])
        b2_sb = const.tile([128, 2, 1], F32)
        nc.scalar.dma_start(out=b2_sb, in_=b2[:])
        w3_sb = const.tile([128, 2, 1024], F16)
        nc.scalar.dma_start(out=w3_sb, in_=w3[:])
        b3_sb = const.tile([128, 8, 1], F32)
        nc.scalar.dma_start(out=b3_sb, in_=b3[:])
        gy_sb = const.tile([128, NG, 9], F32)
        nc.scalar.dma_start(out=gy_sb, in_=gy[:])
        gx_sb = const.tile([128, NG, 9], F32)
        nc.scalar.dma_start(out=gx_sb, in_=gx[:])
        gq_sb = const.tile([128, NG, 9], F32)
        nc.scalar.dma_start(out=gq_sb, in_=gq[:])
        id16 = const.tile([128, 128], F16)
        make_identity(nc, id16)
        id32 = const.tile([128, 128], F32)
        make_identity(nc, id32)
        sel_sb = const.tile([36, 36, 128], F16)
        nc.scalar.dma_start(out=sel_sb, in_=sel[:])
        esel_sb = const.tile([128, 8, 16], F32)
        nc.scalar.dma_start(out=esel_sb, in_=esel[:])

        DUMMY_GATHER = True
        if DUMMY_GATHER:
            # dummy 128-idx gather: forces the Pool SWDGE library load early
            idx0 = const.tile([16, 8], I16)
            nc.gpsimd.memset(idx0[:], 0)
            with tc.tile_pool(name="gdum", bufs=1) as gdp:
                gdum = gdp.tile([128, 2, 128], F16)
                dum_ap = bass.AP(tensor=o1t2[:].tensor, offset=0,
                                 ap=[[256, 64], [1, 256]])
                nc.gpsimd.dma_gather(
                    out_ap=gdum, in_ap=dum_ap, idxs_ap=idx0,
                    num_idxs=128, num_idxs_reg=128,
                    elem_size=256, elem_step=256, transpose=True,
                    single_packet=False)

        # whole-kernel working tensors
        out1_sb = wk.tile([128, 2, SLABPIX], F16)
        offT = wk.tile([128, NG, 18], F32)
        W4 = wk.tile([128, NG, 9, 4], F32)
        W4C = wk.tile([36, NPIX], F16)
        idx16 = wk.tile([16, NIDX // 16], I16)
        idx_sb = wk.tile([128, NIDX // 16], I16)
        stg = wk.tile([128, NGT, 256], F16)

        # o1t2 token writes: row y lands twice (s=0 of block y+1: t=2q+132;
        # s=1 of block y: t=2q+1). Split per half so writes start early.
        NFULL = SLABPIX // 128          # 18 full transpose chunks
        TAILW = SLABPIX - NFULL * 128   # 72
        o1flat = o1t2[:].rearrange("t c -> (t c)")
        wr_insts = []

        def o1t2_write(c0, c1, eng):
            for toff in (132 * 256, 256):
                dstm = bass.AP(tensor=o1flat.tensor,
                               offset=toff + 512 * 128 * c0,
                               ap=[[512, 128], [512 * 128, c1 - c0], [1, 256]])
                wr = eng.dma_start(out=dstm, in_=stg[:, c0:c1, :])
                wr_insts.append(wr.ins)

        def o1t2_tail(eng):
            for toff in (132 * 256, 256):
                dstt = bass.AP(tensor=o1flat.tensor,
                               offset=toff + 512 * NFULL * 128,
                               ap=[[512, TAILW], [1, 256]])
                wr = eng.dma_start(out=dstt, in_=stg[0:TAILW, NFULL, :])
                wr_insts.append(wr.ins)

        def eng_of(i):
            return nc.sync if i % 2 == 0 else nc.scalar

        # ---- phase 1: conv1x1 (1024->256) + BN + ReLU, DMA-pipelined,
        #      interleaved with phase 5 transposes (o1t2 staging)
        NCH = 6
        CW = SLABPIX // NCH  # 396
        with tc.tile_pool(name="psA", bufs=1, space="PSUM") as psA, \
             tc.tile_pool(name="ps5", bufs=2, space="PSUM") as ps5:
            for half in range(2):
                pss = [psA.tile([128, CW], F32, tag=f"c1_{i}",
                                name=f"psc1_{i}") for i in range(6)]
                for kc in range(8):
                    xsb = x_t[kc]
                    for m in range(2):
                        for i in range(3):
                            nch = half * 3 + i
                            sl = slice(nch * CW, (nch + 1) * CW)
                            nc.tensor.matmul(
                                pss[m * 3 + i],
                                lhsT=w1_sb[:, kc, m * 128:(m + 1) * 128],
                                rhs=xsb[:, sl],
                                start=(kc == 0), stop=False)
                for m in range(2):
                    for i in range(3):
                        nch = half * 3 + i
                        sl = slice(nch * CW, (nch + 1) * CW)
                        nc.tensor.matmul(
                            pss[m * 3 + i],
                            lhsT=b1_sb[:1, m * 128:(m + 1) * 128],
                            rhs=msk_sb[:1, sl], start=False, stop=True)
                        nc.scalar.activation(out1_sb[:, m, sl],
                                             pss[m * 3 + i], AF.Relu)
                # phase 5 transposes for the chunks this half completed
                clo = 0 if half == 0 else 9
                chi = 9 if half == 0 else NGT
                for g in range(clo, chi):
                    wc = min(128, SLABPIX - g * 128)
                    pt = ps5.tile([128, 256], F16, tag="t16", name="pt5")
                    for kc in range(2):
                        nc.tensor.transpose(
                            pt[:wc, kc * 128:(kc + 1) * 128],
                            out1_sb[:, kc, g * 128:g * 128 + wc], id16)
                    nc.vector.tensor_copy(stg[:wc, g, :], pt[:wc, :])
                if half == 0:
                    o1t2_write(0, 9, nc.sync)
                else:
                    # chunk 9 split out: batch-0 gathers only touch tokens
                    # of pixels < 1280, i.e. chunks 0-9 (wr_insts[:4])
                    o1t2_write(9, 10, nc.sync)
                    o1t2_write(10, NFULL, nc.sync)
                    o1t2_tail(nc.sync)

        xpool_cm.__exit__(None, None, None)

        # ---- phases 2-4 PER BATCH: offset conv -> transpose -> coords ->
        #      index fold -> idx broadcast. Batch 0's chain completes first
        #      so its gathers start while batch 1's chain still runs.
        #      idx layout: slot = 288*b + 32*t + 8*g' + a (all contiguous
        #      per (b, t) so every DMA is a plain 2D copy).
        oy = offT[:, :, 0:9]
        ox = offT[:, :, 9:18]
        I32 = mybir.dt.int32

        def stile(tag):
            return small.tile([128, NG, 9], F32, tag=tag, name=tag)

        py, px, u, v, qf, qf2 = (stile(t) for t in
                                 ("py", "px", "u", "v", "qf", "qf2"))
        fs = {}
        for pfx in ("y", "x"):
            fs[pfx] = dict(
                ii=small.tile([128, NG, 9], I32, tag=pfx + "i",
                              name=pfx + "i"),
                fc=stile(pfx + "c"), d=stile(pfx + "d"), mk=stile(pfx + "m"),
                fl=stile(pfx + "f"), fr=stile(pfx + "r"))

        def floorsplit_b(p, pfx, bsl):
            """Exact floor via cast roundtrip + negative-error fixup."""
            t = fs[pfx]
            V = nc.vector
            V.tensor_copy(t['ii'][:, bsl], p[:, bsl])
            V.tensor_copy(t['fc'][:, bsl], t['ii'][:, bsl])
            V.tensor_tensor(t['d'][:, bsl], p[:, bsl], t['fc'][:, bsl],
                            ALU.subtract)
            V.tensor_scalar(t['mk'][:, bsl], t['d'][:, bsl], 0.0, None,
                            ALU.is_lt)
            V.tensor_tensor(t['fl'][:, bsl], t['fc'][:, bsl],
                            t['mk'][:, bsl], ALU.subtract)
            V.tensor_tensor(t['fr'][:, bsl], p[:, bsl], t['fl'][:, bsl],
                            ALU.subtract)
            return t['fl'], t['fr']

        with tc.tile_pool(name="bpool", bufs=1) as bpool, \
             tc.tile_pool(name="psB", bufs=2, space="PSUM") as psB, \
             tc.tile_pool(name="ps3", bufs=2, space="PSUM") as ps3, \
             tc.tile_pool(name="psI", bufs=2, space="PSUM") as psIp, \
             tc.tile_pool(name="ps4", bufs=2, space="PSUM") as ps4:
            offs_sb = bpool.tile([18, 1024], F32)
            for b in range(B):
                bsl = slice(4 * b, 4 * b + 4)
                for hh in range(2):
                    base = (b * RS + MH) * WP + hh * 264
                    ps = psB.tile([18, 264], F32, tag="off", name="psoff")
                    first = True
                    for t in range(9):
                        tau = (t // 3 - 1) * WP + (t % 3 - 1)
                        for kc in range(2):
                            nc.tensor.matmul(
                                ps, lhsT=woff_sb[:, t, kc, :],
                                rhs=out1_sb[:, kc,
                                            base + tau:base + tau + 264],
                                start=first,
                                stop=(t == 8 and kc == 1))
                            first = False
                    dst = offs_sb[:, (b * 2 + hh) * 256:
                                  (b * 2 + hh + 1) * 256].rearrange(
                        "p (r c) -> p r c", c=64)
                    src = ps.rearrange("p (r c) -> p r c",
                                       c=WP)[:, :, 1:65]
                    nc.scalar.activation(dst, src, AF.Identity,
                                         bias=boff_sb)

                # offsets -> pixel-major for this batch's groups
                for g in range(4 * b, 4 * b + 4):
                    p32 = ps3.tile([128, 18], F32, tag="t32", name="p32")
                    nc.tensor.transpose(
                        p32, offs_sb[:, g * 128:(g + 1) * 128],
                        id32[:18, :18])
                    nc.vector.tensor_copy(offT[:, g, :], p32)

                # coords + bilinear weights (slab rows clamp [0, RS-2.1])
                V = nc.vector
                V.tensor_tensor(py[:, bsl], oy[:, bsl], gy_sb[:, bsl],
                                ALU.add)
                V.tensor_scalar(py[:, bsl], py[:, bsl], 0.0,
                                float(RS) - 2.1, ALU.max, ALU.min)
                y0f, fy = floorsplit_b(py, "y", bsl)
                V.tensor_tensor(px[:, bsl], ox[:, bsl], gx_sb[:, bsl],
                                ALU.add)
                V.tensor_scalar(px[:, bsl], px[:, bsl], 0.0, 65.9,
                                ALU.max, ALU.min)
                x0f, fx = floorsplit_b(px, "x", bsl)
                V.tensor_scalar(u[:, bsl], fy[:, bsl], -1.0, 1.0,
                                ALU.mult, ALU.add)
                V.tensor_scalar(v[:, bsl], fx[:, bsl], -1.0, 1.0,
                                ALU.mult, ALU.add)
                V.tensor_tensor(W4[:, bsl, :, 0], u[:, bsl], v[:, bsl],
                                ALU.mult)
                V.tensor_tensor(W4[:, bsl, :, 1], u[:, bsl], fx[:, bsl],
                                ALU.mult)
                V.tensor_tensor(W4[:, bsl, :, 2], fy[:, bsl], v[:, bsl],
                                ALU.mult)
                V.tensor_tensor(W4[:, bsl, :, 3], fy[:, bsl], fx[:, bsl],
                                ALU.mult)

                # token index: q = 132*y0 + 2*x0 + (2376*b + 132)
                V.scalar_tensor_tensor(qf[:, bsl], y0f[:, bsl], 132.0,
                                       gq_sb[:, bsl], ALU.mult, ALU.add)
                V.scalar_tensor_tensor(qf2[:, bsl], x0f[:, bsl], 2.0,
                                       qf[:, bsl], ALU.mult, ALU.add)

                # partition fold 128->16 via one-hot fp32 matmuls (exact):
                # psI[mm, 36a + 9g' + k] = qf2[16a+mm, 4b+g', k] then one
                # DVE permute-cast into idx16[:, 288b + 32t + 8g' + a]
                psI = psIp.tile([16, 288], F32, tag="psI", name=f"psI{b}")
                qf2f = qf2.rearrange("p g k -> p (g k)")
                for a in range(8):
                    nc.tensor.matmul(psI[:, a * 36:(a + 1) * 36],
                                     lhsT=esel_sb[:, a, :],
                                     rhs=qf2f[:, 36 * b:36 * b + 36],
                                     start=True, stop=True)
                nc.vector.tensor_copy(
                    idx16[:, 288 * b:288 * b + 288].rearrange(
                        "m (k g a) -> m k g a", k=9, g=4, a=8),
                    psI.rearrange("m (a g k) -> m k g a", a=8, g=4, k=9))

                # broadcast this batch's idx block (contiguous 2D copies)
                for bb in range(8):
                    eng_of(bb + b).dma_start(
                        out=idx_sb[16 * bb:16 * bb + 16,
                                   288 * b:288 * b + 288],
                        in_=idx16[:, 288 * b:288 * b + 288])

                # W4 -> channel-major fp16 rows for this batch's groups
                for g in range(4 * b, 4 * b + 4):
                    p36 = ps4.tile([36, 128], F32, tag="w4c", name="p36")
                    nc.tensor.transpose(
                        p36, W4[:, g, :, :].rearrange("p a b -> p (a b)"),
                        id32)
                    nc.scalar.copy(W4C[:, g * 128:(g + 1) * 128], p36)

        # residual pixels preloaded from xs HBM (overlaps phase 6)
        xr_t = []
        for m in range(8):
            xsrc4 = xs[:, m, :].rearrange("p (b r c) -> p b r c", r=RS, c=WP)
            xr = wk.tile([128, B, RPC, 64], F16, tag=f"xr{m}", name=f"xr{m}")
            for b in range(B):
                eng_of(m + b).dma_start(
                    out=xr[:, b], in_=xsrc4[:, b, MH:MH + RPC, 1:65])
            xr_t.append(xr)

        # ---- phase 6: transposed patch gather (channel-major) + weighting
        src_ap = bass.AP(tensor=o1t2[:].tensor, offset=0,
                         ap=[[256, O1TOK - 3], [1, 1024]])
        psd_cm = tc.tile_pool(name="psd", bufs=2, space="PSUM")
        with tc.tile_pool(name="gpool", bufs=3) as gpool, \
             tc.tile_pool(name="wpool", bufs=3) as wpool, \
             tc.tile_pool(name="wps", bufs=2, space="PSUM") as wps, \
             tc.tile_pool(name="accp", bufs=1) as accp, \
             tc.tile_pool(name="spool", bufs=2) as spool, \
             psd_cm as psd:
            out2_sb = wk.tile([128, 2, NPIX], F16)

            # pixel-half (== batch) software pipeline: half-0 taps, then
            # half-0 conv3/out overlapped with half-1 taps. Broadcast weight
            # rows (sel-matmul + ACT evac) are prefetched per half.
            nidx_reg = nc.gpsimd.to_reg(512)
            with tc.tile_pool(name="psC", bufs=2, space="PSUM") as psC:
                for h in range(2):
                    hs = slice(h * 512, (h + 1) * 512)
                    wB_t = []
                    for t in range(9):
                        wB = wpool.tile([128, 4, 512], F16, tag="wB",
                                        name=f"wB{h}_{t}")
                        for nb in range(4):
                            wp = wps.tile([128, 512], F32, tag="wp",
                                          name="wp")
                            nc.tensor.matmul(
                                wp, lhsT=sel_sb[:, 4 * t + nb, :],
                                rhs=W4C[:, hs], start=True, stop=True)
                            nc.scalar.copy(wB[:, nb, :], wp)
                        wB_t.append(wB)
                    dps = psd.tile([128, 2, 512], F32, tag="dps", name="dps")
                    for t in range(9):
                        wB = wB_t[t]
                        g_t = gpool.tile([128, 8, 512], F16, tag="g",
                                         name="g_t")
                        gi = nc.gpsimd.dma_gather(
                            out_ap=g_t, in_ap=src_ap,
                            idxs_ap=idx_sb[:, 288 * h + 32 * t:
                                           288 * h + 32 * t + 32],
                            num_idxs=512, num_idxs_reg=nidx_reg,
                            elem_size=1024, elem_step=256, transpose=True,
                            single_packet=False)
                        deps = wr_insts[:4] if h == 0 else wr_insts
                        for wi in deps:
                            add_dep_helper(gi.ins, wi,
                                           reason="gather after o1t2")

                        # chunks: (0,1)=y0x0 (2,3)=y1x0 (4,5)=y0x1 (6,7)=y1x1
                        # nb: 0=u*v(y0x0) 1=u*fx(y0x1) 2=fy*v(y1x0) 3=fy*fx
                        def wv(nb):
                            return wB[:, nb:nb + 1, :].broadcast_to(
                                [128, 2, 512])

                        ta = accp.tile([128, 2, 512], F16, tag="ta",
                                       name="ta")
                        nc.vector.tensor_tensor(ta, g_t[:, 0:2, :], wv(0),
                                                ALU.mult)
                        tb = accp.tile([128, 2, 512], F16, tag="tb",
                                       name="tb")
                        nc.vector.tensor_tensor(tb, g_t[:, 2:4, :], wv(2),
                                                ALU.mult)
                        tab = accp.tile([128, 2, 512], F16, tag="tab",
                                        name="tab")
                        nc.vector.tensor_tensor(tab, ta, tb, ALU.add)
                        tc_ = accp.tile([128, 2, 512], F16, tag="ta",
                                        name="tc_")
                        nc.vector.tensor_tensor(tc_, g_t[:, 4:6, :], wv(1),
                                                ALU.mult)
                        td = accp.tile([128, 2, 512], F16, tag="tb",
                                       name="td")
                        nc.vector.tensor_tensor(td, g_t[:, 6:8, :], wv(3),
                                                ALU.mult)
                        tcd = accp.tile([128, 2, 512], F16, tag="tcd",
                                        name="tcd")
                        nc.vector.tensor_tensor(tcd, tc_, td, ALU.add)
                        S = spool.tile([128, 2, 512], F16, tag="S", name="S")
                        nc.vector.tensor_tensor(S, tab, tcd, ALU.add)

                        for ch in range(2):
                            j = 2 * t + ch
                            for m in range(2):
                                nc.tensor.matmul(
                                    dps[:, m, :],
                                    lhsT=w2_sb[:, j, m * 128:(m + 1) * 128],
                                    rhs=S[:, ch, :],
                                    start=(t == 0 and ch == 0),
                                    stop=(t == 8 and ch == 1))

                    # deform psum evac for this half: BN + ReLU
                    for m2 in range(2):
                        nc.scalar.activation(out2_sb[:, m2, hs],
                                             dps[:, m2, :],
                                             AF.Relu, bias=b2_sb[:, m2, :])

                    # conv3 + residual + out for this half (overlaps the
                    # other half's gathers)
                    for m in range(8):
                        ps = psC.tile([128, 512], F32, tag="c3", name="psc3")
                        for kc in range(2):
                            nc.tensor.matmul(
                                ps,
                                lhsT=w3_sb[:, kc, m * 128:(m + 1) * 128],
                                rhs=out2_sb[:, kc, hs],
                                start=(kc == 0), stop=(kc == 1))
                        xv = xr_t[m].rearrange("p b r c -> p (b r c)")[:, hs]
                        rt = opool.tile([128, 512], F32, tag="res",
                                        name="rt")
                        nc.vector.tensor_tensor(rt, ps, xv, ALU.add)
                        ot = opool.tile([128, 512], F16, tag="out",
                                        name="ot")
                        nc.scalar.activation(ot, rt, AF.Relu,
                                             bias=b3_sb[:, m, :])
                        eng_of(m).dma_start(out=outp[m][:, hs], in_=ot)

    nc.compile()
    return nc


# ---------------------------------------------------------------- host side
def fold_weights(inputs):
    f = {}
    w1, g1, b1, m1, v1 = (np.asarray(inputs[k], np.float64)
                          for k in ("w1", "g1", "b1", "m1", "v1"))
    s1 = g1 / np.sqrt(v1 + EPS)
    W1f = w1[:, :, 0, 0] * s1[:, None]
    f["w1"] = np.ascontiguousarray(
        W1f.T.reshape(8, 128, 256).transpose(1, 0, 2)).astype(np.float16)
    f["b1"] = (b1 - m1 * s1).reshape(1, 256).astype(np.float16)

    w_off = np.asarray(inputs["w_off"], np.float64)
    b_off = np.asarray(inputs["b_off"], np.float64)
    perm = np.concatenate([np.arange(0, 18, 2), np.arange(1, 18, 2)])
    wofft = np.zeros((9, 2, 128, 18), np.float16)
    for t in range(9):
        wk = w_off[perm, :, t // 3, t % 3].T  # [256, 18]
        wofft[t] = wk.reshape(2, 128, 18).astype(np.float16)
    f["woff"] = np.ascontiguousarray(wofft.transpose(2, 0, 1, 3))
    f["boff"] = b_off[perm].reshape(18, 1).astype(np.float32)

    w2, g2, b2, m2, v2 = (np.asarray(inputs[k], np.float64)
                          for k in ("w2", "g2", "b2", "m2", "v2"))
    s2 = g2 / np.sqrt(v2 + EPS)
    W2f = w2 * s2[:, None, None, None]
    w2r = np.zeros((18, 128, 256), np.float16)
    for t in range(9):
        for ch in range(2):
            w2r[2 * t + ch] = W2f[:, ch * 128:(ch + 1) * 128,
                                  t // 3, t % 3].T.astype(np.float16)
    f["w2"] = np.ascontiguousarray(w2r.transpose(1, 0, 2))
    f["b2"] = np.ascontiguousarray(
        (b2 - m2 * s2).reshape(2, 128, 1).transpose(1, 0, 2)).astype(np.float32)

    w3, g3, b3, m3, v3 = (np.asarray(inputs[k], np.float64)
                          for k in ("w3", "g3", "b3", "m3", "v3"))
    s3 = g3 / np.sqrt(v3 + EPS)
    W3f = w3[:, :, 0, 0] * s3[:, None]
    f["w3"] = np.ascontiguousarray(
        W3f.T.reshape(2, 128, 1024).transpose(1, 0, 2)).astype(np.float16)
    f["b3"] = np.ascontiguousarray(
        (b3 - m3 * s3).reshape(8, 128, 1).transpose(1, 0, 2)).astype(np.float32)
    return f


def make_grids():
    p = np.arange(NPIX)
    b = p // (RPC * W)
    r = (p % (RPC * W)) // W
    c = p % W
    kdy = np.arange(9) // 3 - 1
    kdx = np.arange(9) % 3 - 1
    gy = (r + MH)[:, None] + kdy[None, :]          # batch-relative slab row
    gx = (c + 1)[:, None] + kdx[None, :]
    gq = np.broadcast_to((SLABPIX // B * 2 * b + 132)[:, None],
                         gy.shape).copy()          # 2376*b + 132

    def shape(a):
        return np.ascontiguousarray(
            a.reshape(NG, 128, 9).transpose(1, 0, 2)).astype(np.float32)
    return shape(gy), shape(gx), shape(gq)


def make_core_inputs(x, folded, grids, core):
    row0 = RPC * core - MH
    xp = np.zeros((B, CIN, RS, WP), np.float16)
    lo, hi = max(0, row0), min(H, row0 + RS)
    if hi > lo:
        xp[:, :, lo - row0:hi - row0, 1:65] = x[:, :, lo:hi, :].astype(np.float16)
    xs = np.ascontiguousarray(
        xp.transpose(1, 0, 2, 3).reshape(8, 128, SLABPIX).transpose(1, 0, 2))

    mk = np.zeros((B, RS, WP), np.float16)
    mk[:, lo - row0:hi - row0, 1:65] = 1.0

    gy, gx, gq = grids
    selmat = np.zeros((36, 36, 128), np.float16)
    for k in range(36):
        selmat[k, k, :] = 1.0
    eselmat = np.zeros((128, 8, 16), np.float32)
    for a in range(8):
        for mm in range(16):
            eselmat[16 * a + mm, a, mm] = 1.0
    m = dict(folded)
    m.update(xs=xs, msk=mk.reshape(1, SLABPIX), gy=gy, gx=gx, gq=gq,
             sel=selmat, esel=eselmat)
    return m


def assemble_output(results):
    full = np.zeros((B, COUT, H, W), np.float32)
    for core, res in enumerate(results):
        o = np.asarray(res["out"]).reshape(8, 128, B, RPC, W)
        full[:, :, RPC * core:RPC * (core + 1), :] = o.transpose(
            2, 0, 1, 3, 4).reshape(B, COUT, RPC, W)
    return full


_NC_CACHE = None


def kernel(**inputs):
    global _NC_CACHE
    from concourse.bass_utils import run_bass_kernel_spmd

    x = np.asarray(inputs["x"], np.float32)
    folded = fold_weights(inputs)
    grids = make_grids()
    in_maps = [make_core_inputs(x, folded, grids, i) for i in range(NCORES)]

    if _NC_CACHE is None:
        _NC_CACHE = build_nc()
    res = run_bass_kernel_spmd(_NC_CACHE, in_maps, list(range(NCORES)))
    return assemble_output(res.results)

